# revision 3
# baseline (speedup 1.0000x reference)
"""Multi-head attention (B=2, S=2048, D=1024, H=16) on 8 Trainium2 cores.

Sharding: tensor-parallel over heads x data-parallel over batch.
Core c handles batch b = c//4 and heads 4*(c%4) .. 4*(c%4)+3.

Per-core dataflow (all matmuls float32r unless noted):
  Phase A: Q^T = (Wq_c @ x_q^T)  [256, 2048]   (do-major, bias via DVE evict)
           K^T same; V = x_v @ Wv_c^T [2048, 256] (bias via ones-row in x_v^T)
  Phase B: per (head h, q-block of 128 rows):
           scores = Q_h^T.T @ K_h^T -> psum [128q, 2048k]
           expS   = exp(scores/8) via ACT evict, accum_out -> denom
           attn   = expS * (1/denom)  (gpsimd)  -> DMA out (fp32)
           attnT  = PE-transpose of attn tiles -> DVE evict into k-major slab
           ctx^T_h[64, q] += V_h[kb].T-contract attnT  (per 256-q superblock)
  Phase C: out[q, o] = ctx^T.T @ Wo_c^T  [2048, 1024] partial, host-summed.

Host: shards/transposes inputs, gathers attn heads, sums out partials + bo.
"""

import numpy as np
from contextlib import ExitStack

import concourse.bacc as bacc
import concourse.tile as tile
import concourse.mybir as mybir
from concourse.bass_utils import run_bass_kernel_spmd
from concourse.masks import make_identity

f32 = mybir.dt.float32
f32r = mybir.dt.float32r
AFT = mybir.ActivationFunctionType

B, S, D, H = 2, 2048, 1024, 16
HD = D // H            # 64 head dim
NCORES = 8
GPB = NCORES // B      # head-groups per batch = 4
HPC = H // GPB         # heads per core = 4
DV = HPC * HD          # 256 = per-core projection width
SCALE = 1.0 / 8.0      # 1/sqrt(HD)

_compiled = None


def _build():
    nc = bacc.Bacc("TRN2", target_bir_lowering=False, debug=False,
                   num_devices=NCORES)
    xqT = nc.dram_tensor("xqT", [D, S], f32r, kind="ExternalInput")
    xkT = nc.dram_tensor("xkT", [D, S], f32r, kind="ExternalInput")
    xvT = nc.dram_tensor("xvT", [D + 1, S], f32r, kind="ExternalInput")
    wqT = nc.dram_tensor("wqT", [D, DV], f32r, kind="ExternalInput")
    wkT = nc.dram_tensor("wkT", [D, DV], f32r, kind="ExternalInput")
    wvT = nc.dram_tensor("wvT", [D + 1, DV], f32r, kind="ExternalInput")
    woT = nc.dram_tensor("woT", [DV, D], f32r, kind="ExternalInput")
    bq = nc.dram_tensor("bq", [DV, 1], f32, kind="ExternalInput")
    bk = nc.dram_tensor("bk", [DV, 1], f32, kind="ExternalInput")
    attn4 = nc.dram_tensor("attn4", [HPC, S, S], f32, kind="ExternalOutput")
    outp = nc.dram_tensor("outp", [S, D], f32, kind="ExternalOutput")

    with tile.TileContext(nc) as tc, ExitStack() as ctx:
        const = ctx.enter_context(tc.tile_pool(name="const", bufs=1))
        ident = const.tile([128, 128], f32)
        make_identity(nc, ident[:])

        wpool = ctx.enter_context(tc.tile_pool(name="w", bufs=1))
        wq_t, wk_t, wv_t = [], [], []
        for i in range(8):
            t = wpool.tile([128, DV], f32r, tag=f"wq{i}")
            nc.sync.dma_start(t[:], wqT[i * 128:(i + 1) * 128, :])
            wq_t.append(t)
            t = wpool.tile([128, DV], f32r, tag=f"wk{i}")
            nc.sync.dma_start(t[:], wkT[i * 128:(i + 1) * 128, :])
            wk_t.append(t)
            t = wpool.tile([128, DV], f32r, tag=f"wv{i}")
            nc.sync.dma_start(t[:], wvT[i * 128:(i + 1) * 128, :])
            wv_t.append(t)
        wv_last = wpool.tile([1, DV], f32r, tag="wvlast")
        nc.sync.dma_start(wv_last[:], wvT[D:D + 1, :])
        wo_t = []
        for i in range(2):
            t = wpool.tile([128, D], f32r, tag=f"wo{i}")
            nc.sync.dma_start(t[:], woT[i * 128:(i + 1) * 128, :])
            wo_t.append(t)
        bq_t, bk_t = [], []
        for i in range(2):
            t = wpool.tile([128, 1], f32, tag=f"bq{i}")
            nc.sync.dma_start(t[:], bq[i * 128:(i + 1) * 128, :])
            bq_t.append(t)
            t = wpool.tile([128, 1], f32, tag=f"bk{i}")
            nc.sync.dma_start(t[:], bk[i * 128:(i + 1) * 128, :])
            bk_t.append(t)

        proj = ctx.enter_context(tc.tile_pool(name="proj", bufs=1))
        qt_t = [proj.tile([128, S], f32r, tag=f"qt{i}", name=f"qt{i}") for i in range(2)]
        kt_t = [proj.tile([128, S], f32r, tag=f"kt{i}", name=f"kt{i}") for i in range(2)]
        v_t = [proj.tile([128, DV], f32r, tag=f"v{i}", name=f"v{i}") for i in range(16)]
        ctxT_t = [proj.tile([128, S], f32r, tag=f"ctxT{i}", name=f"ctxT{i}") for i in range(2)]

        # ---------------- Phase A: projections ----------------
        with tc.tile_pool(name="xa", bufs=10) as xpool, \
             tc.tile_pool(name="psA", bufs=2, space="PSUM") as psA:

            def load_x(src, n=8):
                ts = []
                for di in range(n):
                    t = xpool.tile([128, S], f32r, tag="x")
                    nc.sync.dma_start(t[:], src[di * 128:(di + 1) * 128, :])
                    ts.append(t)
                return ts

            # Q^T and K^T projections: [do 256, s 2048]
            for w_t, x_src, b_t, o_t in ((wq_t, xqT, bq_t, qt_t),
                                         (wk_t, xkT, bk_t, kt_t)):
                x_t = load_x(x_src)
                for do in range(2):
                    for sc in range(4):
                        ps = psA.tile([128, 512], f32, tag="p")
                        for di in range(8):
                            nc.tensor.matmul(
                                ps[:],
                                w_t[di][:, do * 128:(do + 1) * 128],
                                x_t[di][:, sc * 512:(sc + 1) * 512],
                                start=(di == 0), stop=(di == 7))
                        nc.vector.tensor_scalar_add(
                            o_t[do][:, sc * 512:(sc + 1) * 512], ps[:],
                            b_t[do][:])
            # V projection: [s 2048, dv 256], bias via ones-row (di index 8)
            xv_t = load_x(xvT)
            xv_last = xpool.tile([1, S], f32r, tag="xlast", bufs=1)
            nc.sync.dma_start(xv_last[:], xvT[D:D + 1, :])
            for sb in range(16):
                ps = psA.tile([128, DV], f32, tag="pv")
                for di in range(8):
                    nc.tensor.matmul(
                        ps[:], xv_t[di][:, sb * 128:(sb + 1) * 128],
                        wv_t[di][:], start=(di == 0), stop=False)
                nc.tensor.matmul(
                    ps[:], xv_last[:, sb * 128:(sb + 1) * 128], wv_last[:],
                    start=False, stop=True)
                nc.vector.tensor_copy(v_t[sb][:], ps[:])

        # ---------------- Phase B: attention ----------------
        with tc.tile_pool(name="eS", bufs=3) as epool, \
             tc.tile_pool(name="aT", bufs=4) as apool, \
             tc.tile_pool(name="slab", bufs=2) as slabpool, \
             tc.tile_pool(name="dn", bufs=6) as dpool, \
             tc.tile_pool(name="psS", bufs=1, space="PSUM") as psS, \
             tc.tile_pool(name="psT", bufs=1, space="PSUM") as psT, \
             tc.tile_pool(name="psC", bufs=2, space="PSUM") as psC:
            for h in range(HPC):
                ht, ho = h // 2, 64 * (h % 2)
                for qsb in range(8):           # 256-q superblocks
                    slab = slabpool.tile([128, 16 * 256], f32r, tag="slab")
                    for qq in range(2):        # 128-q blocks within
                        qb = qsb * 2 + qq
                        ps_s = psS.tile([128, 2048], f32, tag="s")
                        for kc in range(4):
                            nc.tensor.matmul(
                                ps_s[:, kc * 512:(kc + 1) * 512],
                                qt_t[ht][ho:ho + 64, qb * 128:(qb + 1) * 128],
                                kt_t[ht][ho:ho + 64, kc * 512:(kc + 1) * 512],
                                start=True, stop=True)
                        eS = epool.tile([128, 2048], f32, tag="e")
                        dn = dpool.tile([128, 1], f32, tag="d")
                        nc.scalar.activation(eS[:], ps_s[:], AFT.Exp,
                                             scale=SCALE, accum_out=dn[:])
                        rc = dpool.tile([128, 1], f32, tag="r")
                        nc.vector.reciprocal(rc[:], dn[:])
                        at_t = apool.tile([128, 2048], f32, tag="a")
                        nc.gpsimd.tensor_scalar_mul(at_t[:], eS[:], rc[:])
                        nc.sync.dma_start(
                            attn4[h, qb * 128:(qb + 1) * 128, :], at_t[:])
                        # transpose attn -> k-major slab
                        for tg in range(2):
                            ps_t = psT.tile([128, 1024], f32, tag="t")
                            for u in range(8):
                                tb = tg * 8 + u
                                nc.tensor.transpose(
                                    ps_t[:, u * 128:(u + 1) * 128],
                                    at_t[:, tb * 128:(tb + 1) * 128],
                                    ident[:])
                            dst = slab[:].rearrange(
                                "p (a b) -> p a b", a=16)[
                                :, tg * 8:(tg + 1) * 8,
                                qq * 128:qq * 128 + 128]
                            src = ps_t[:].rearrange("p (a b) -> p a b", a=8)
                            nc.vector.tensor_copy(dst, src)
                    # ctx^T accumulation for this (h, qsb)
                    ps_c = psC.tile([64, 256], f32, tag="c")
                    for kb in range(16):
                        nc.tensor.matmul(
                            ps_c[:], v_t[kb][:, h * 64:(h + 1) * 64],
                            slab[:, kb * 256:(kb + 1) * 256],
                            start=(kb == 0), stop=(kb == 15))
                    nc.vector.tensor_copy(
                        ctxT_t[ht][ho:ho + 64, qsb * 256:(qsb + 1) * 256],
                        ps_c[:])

        # ---------------- Phase C: output projection ----------------
        with tc.tile_pool(name="po", bufs=3) as opool, \
             tc.tile_pool(name="psO", bufs=2, space="PSUM") as psO:
            for qb in range(16):
                ob = opool.tile([128, D], f32, tag="o")
                for oc in range(2):
                    ps_o = psO.tile([128, 512], f32, tag="po")
                    for dvb in range(2):
                        nc.tensor.matmul(
                            ps_o[:],
                            ctxT_t[dvb][:, qb * 128:(qb + 1) * 128],
                            wo_t[dvb][:, oc * 512:(oc + 1) * 512],
                            start=(dvb == 0), stop=(dvb == 1))
                    nc.vector.tensor_copy(ob[:, oc * 512:(oc + 1) * 512],
                                          ps_o[:])
                nc.sync.dma_start(outp[qb * 128:(qb + 1) * 128, :], ob[:])

    nc.compile()
    return nc


def kernel(query, key, value, Wq, bq, Wk, bk, Wv, bv, Wo, bo, **trace_kw):
    global _compiled
    if _compiled is None:
        _compiled = _build()
    nc = _compiled

    query = np.asarray(query, np.float32)
    key = np.asarray(key, np.float32)
    value = np.asarray(value, np.float32)
    Wq, Wk, Wv, Wo = (np.asarray(w, np.float32) for w in (Wq, Wk, Wv, Wo))
    bq, bk, bv, bo = (np.asarray(x, np.float32) for x in (bq, bk, bv, bo))

    ones = np.ones((1, S), np.float32)
    xqT_b = [np.ascontiguousarray(query[b].T) for b in range(B)]
    xkT_b = [np.ascontiguousarray(key[b].T) for b in range(B)]
    xvT_b = [np.concatenate([value[b].T, ones], axis=0) for b in range(B)]

    in_maps = []
    for c in range(NCORES):
        b, hg = c // GPB, c % GPB
        sl = slice(hg * DV, (hg + 1) * DV)
        in_maps.append({
            "xqT": xqT_b[b], "xkT": xkT_b[b], "xvT": xvT_b[b],
            "wqT": np.ascontiguousarray(Wq[sl, :].T),
            "wkT": np.ascontiguousarray(Wk[sl, :].T),
            "wvT": np.concatenate(
                [np.ascontiguousarray(Wv[sl, :].T), bv[sl][None, :]], axis=0),
            "woT": np.ascontiguousarray(Wo[:, sl].T),
            "bq": np.ascontiguousarray(bq[sl].reshape(DV, 1)),
            "bk": np.ascontiguousarray(bk[sl].reshape(DV, 1)),
        })

    res = run_bass_kernel_spmd(nc, in_maps, core_ids=list(range(NCORES)),
                               **trace_kw)

    attn = np.empty((B, H, S, S), np.float32)
    out = np.zeros((B, S, D), np.float32)
    for c in range(NCORES):
        b, hg = c // GPB, c % GPB
        attn[b, hg * HPC:(hg + 1) * HPC] = res.results[c]["attn4"]
        out[b] += res.results[c]["outp"]
    out += bo[None, None, :]
    if trace_kw:
        return (out, attn), res
    return out, attn


# revision 4
# speedup vs baseline: 3.5358x; 3.5358x over previous
"""Multi-head attention (B=2, S=2048, D=1024, H=16) on 8 Trainium2 cores.

Sharding: tensor-parallel over heads x data-parallel over batch.
Core c handles batch b = c//4 and heads 4*(c%4) .. 4*(c%4)+3.

Per-core dataflow (all matmuls float32r unless noted):
  Phase A: Q^T = (Wq_c @ x_q^T)  [256, 2048]   (do-major, bias via DVE evict)
           K^T same; V = x_v @ Wv_c^T [2048, 256] (bias via ones-row in x_v^T)
  Phase B: per (head h, q-block of 128 rows):
           scores = Q_h^T.T @ K_h^T -> psum [128q, 2048k]
           expS   = exp(scores/8) via ACT evict, accum_out -> denom
           attn   = expS * (1/denom)  (gpsimd)  -> DMA out (fp32)
           attnT  = PE-transpose of attn tiles -> DVE evict into k-major slab
           ctx^T_h[64, q] += V_h[kb].T-contract attnT  (per 256-q superblock)
  Phase C: out[q, o] = ctx^T.T @ Wo_c^T  [2048, 1024] partial, host-summed.

Host: shards/transposes inputs, gathers attn heads, sums out partials + bo.
"""

import numpy as np
from contextlib import ExitStack

import concourse.bacc as bacc
import concourse.tile as tile
import concourse.mybir as mybir
from concourse.bass_utils import run_bass_kernel_spmd
from concourse.masks import make_identity

f32 = mybir.dt.float32
f32r = mybir.dt.float32r
AFT = mybir.ActivationFunctionType

B, S, D, H = 2, 2048, 1024, 16
HD = D // H            # 64 head dim
NCORES = 8
GPB = NCORES // B      # head-groups per batch = 4
HPC = H // GPB         # heads per core = 4
DV = HPC * HD          # 256 = per-core projection width
SCALE = 1.0 / 8.0      # 1/sqrt(HD)

_compiled = None


def _build():
    nc = bacc.Bacc("TRN2", target_bir_lowering=False, debug=False,
                   num_devices=NCORES)
    xqT = nc.dram_tensor("xqT", [D, S], f32r, kind="ExternalInput")
    xkT = nc.dram_tensor("xkT", [D, S], f32r, kind="ExternalInput")
    xvT = nc.dram_tensor("xvT", [D + 1, S], f32r, kind="ExternalInput")
    wqT = nc.dram_tensor("wqT", [D, DV], f32r, kind="ExternalInput")
    wkT = nc.dram_tensor("wkT", [D, DV], f32r, kind="ExternalInput")
    wvT = nc.dram_tensor("wvT", [D + 1, DV], f32r, kind="ExternalInput")
    woT = nc.dram_tensor("woT", [DV, D], f32r, kind="ExternalInput")
    bq = nc.dram_tensor("bq", [DV, 1], f32, kind="ExternalInput")
    bk = nc.dram_tensor("bk", [DV, 1], f32, kind="ExternalInput")
    attn4 = nc.dram_tensor("attn4", [HPC, S, S], f32, kind="ExternalOutput")
    outp = nc.dram_tensor("outp", [S, D], f32, kind="ExternalOutput")

    with tile.TileContext(nc) as tc, ExitStack() as ctx:
        const = ctx.enter_context(tc.tile_pool(name="const", bufs=1))
        ident = const.tile([128, 128], f32)
        make_identity(nc, ident[:])

        wpool = ctx.enter_context(tc.tile_pool(name="w", bufs=1))
        wq_t, wk_t, wv_t = [], [], []
        for i in range(8):
            t = wpool.tile([128, DV], f32r, tag=f"wq{i}")
            nc.sync.dma_start(t[:], wqT[i * 128:(i + 1) * 128, :])
            wq_t.append(t)
            t = wpool.tile([128, DV], f32r, tag=f"wk{i}")
            nc.sync.dma_start(t[:], wkT[i * 128:(i + 1) * 128, :])
            wk_t.append(t)
            t = wpool.tile([128, DV], f32r, tag=f"wv{i}")
            nc.sync.dma_start(t[:], wvT[i * 128:(i + 1) * 128, :])
            wv_t.append(t)
        wv_last = wpool.tile([1, DV], f32r, tag="wvlast")
        nc.sync.dma_start(wv_last[:], wvT[D:D + 1, :])
        wo_t = []
        for i in range(2):
            t = wpool.tile([128, D], f32r, tag=f"wo{i}")
            nc.sync.dma_start(t[:], woT[i * 128:(i + 1) * 128, :])
            wo_t.append(t)
        bq_t, bk_t = [], []
        for i in range(2):
            t = wpool.tile([128, 1], f32, tag=f"bq{i}")
            nc.sync.dma_start(t[:], bq[i * 128:(i + 1) * 128, :])
            bq_t.append(t)
            t = wpool.tile([128, 1], f32, tag=f"bk{i}")
            nc.sync.dma_start(t[:], bk[i * 128:(i + 1) * 128, :])
            bk_t.append(t)

        proj = ctx.enter_context(tc.tile_pool(name="proj", bufs=1))
        qt_t = [proj.tile([128, S], f32r, tag=f"qt{i}", name=f"qt{i}") for i in range(2)]
        kt_t = [proj.tile([128, S], f32r, tag=f"kt{i}", name=f"kt{i}") for i in range(2)]
        v_t = [proj.tile([128, DV], f32r, tag=f"v{i}", name=f"v{i}") for i in range(16)]
        ctxT_t = [proj.tile([128, S], f32r, tag=f"ctxT{i}", name=f"ctxT{i}") for i in range(2)]

        # ---------------- Phase A: projections ----------------
        with tc.tile_pool(name="xa", bufs=10) as xpool, \
             tc.tile_pool(name="psA", bufs=2, space="PSUM") as psA:

            def load_x(src, n=8):
                ts = []
                for di in range(n):
                    t = xpool.tile([128, S], f32r, tag="x")
                    nc.sync.dma_start(t[:], src[di * 128:(di + 1) * 128, :])
                    ts.append(t)
                return ts

            # Q^T and K^T projections: [do 256, s 2048]
            for w_t, x_src, b_t, o_t in ((wq_t, xqT, bq_t, qt_t),
                                         (wk_t, xkT, bk_t, kt_t)):
                x_t = load_x(x_src)
                for do in range(2):
                    for sc in range(4):
                        ps = psA.tile([128, 512], f32, tag="p")
                        for di in range(8):
                            nc.tensor.matmul(
                                ps[:],
                                w_t[di][:, do * 128:(do + 1) * 128],
                                x_t[di][:, sc * 512:(sc + 1) * 512],
                                start=(di == 0), stop=(di == 7))
                        nc.vector.tensor_scalar_add(
                            o_t[do][:, sc * 512:(sc + 1) * 512], ps[:],
                            b_t[do][:])
            # V projection: [s 2048, dv 256], bias via ones-row (di index 8)
            xv_t = load_x(xvT)
            xv_last = xpool.tile([1, S], f32r, tag="xlast", bufs=1)
            nc.sync.dma_start(xv_last[:], xvT[D:D + 1, :])
            for sb in range(16):
                ps = psA.tile([128, DV], f32, tag="pv")
                for di in range(8):
                    nc.tensor.matmul(
                        ps[:], xv_t[di][:, sb * 128:(sb + 1) * 128],
                        wv_t[di][:], start=(di == 0), stop=False)
                nc.tensor.matmul(
                    ps[:], xv_last[:, sb * 128:(sb + 1) * 128], wv_last[:],
                    start=False, stop=True)
                nc.vector.tensor_copy(v_t[sb][:], ps[:])

        # ---------------- Phase B: attention ----------------
        with tc.tile_pool(name="eS", bufs=3) as epool, \
             tc.tile_pool(name="aT", bufs=4) as apool, \
             tc.tile_pool(name="slab", bufs=2) as slabpool, \
             tc.tile_pool(name="dn", bufs=6) as dpool, \
             tc.tile_pool(name="psS", bufs=1, space="PSUM") as psS, \
             tc.tile_pool(name="psT", bufs=1, space="PSUM") as psT, \
             tc.tile_pool(name="psC", bufs=2, space="PSUM") as psC:
            for h in range(HPC):
                ht, ho = h // 2, 64 * (h % 2)
                for qsb in range(8):           # 256-q superblocks
                    slab = slabpool.tile([128, 16 * 256], f32r, tag="slab")
                    for qq in range(2):        # 128-q blocks within
                        qb = qsb * 2 + qq
                        ps_s = psS.tile([128, 2048], f32, tag="s")
                        for kc in range(4):
                            nc.tensor.matmul(
                                ps_s[:, kc * 512:(kc + 1) * 512],
                                qt_t[ht][ho:ho + 64, qb * 128:(qb + 1) * 128],
                                kt_t[ht][ho:ho + 64, kc * 512:(kc + 1) * 512],
                                start=True, stop=True)
                        eS = epool.tile([128, 2048], f32, tag="e")
                        dn = dpool.tile([128, 1], f32, tag="d")
                        nc.scalar.activation(eS[:], ps_s[:], AFT.Exp,
                                             scale=SCALE, accum_out=dn[:])
                        rc = dpool.tile([128, 1], f32, tag="r")
                        nc.vector.reciprocal(rc[:], dn[:])
                        at_t = apool.tile([128, 2048], f32, tag="a")
                        nc.vector.tensor_scalar_mul(at_t[:], eS[:], rc[:])
                        nc.sync.dma_start(
                            attn4[h, qb * 128:(qb + 1) * 128, :], at_t[:])
                        # transpose attn -> k-major slab
                        for tg in range(2):
                            ps_t = psT.tile([128, 1024], f32, tag="t")
                            for u in range(8):
                                tb = tg * 8 + u
                                nc.tensor.transpose(
                                    ps_t[:, u * 128:(u + 1) * 128],
                                    at_t[:, tb * 128:(tb + 1) * 128],
                                    ident[:])
                            dst = slab[:].rearrange(
                                "p (a b) -> p a b", a=16)[
                                :, tg * 8:(tg + 1) * 8,
                                qq * 128:qq * 128 + 128]
                            src = ps_t[:].rearrange("p (a b) -> p a b", a=8)
                            nc.vector.tensor_copy(dst, src)
                    # ctx^T accumulation for this (h, qsb)
                    ps_c = psC.tile([64, 256], f32, tag="c")
                    for kb in range(16):
                        nc.tensor.matmul(
                            ps_c[:], v_t[kb][:, h * 64:(h + 1) * 64],
                            slab[:, kb * 256:(kb + 1) * 256],
                            start=(kb == 0), stop=(kb == 15))
                    nc.vector.tensor_copy(
                        ctxT_t[ht][ho:ho + 64, qsb * 256:(qsb + 1) * 256],
                        ps_c[:])

        # ---------------- Phase C: output projection ----------------
        with tc.tile_pool(name="po", bufs=3) as opool, \
             tc.tile_pool(name="psO", bufs=2, space="PSUM") as psO:
            for qb in range(16):
                ob = opool.tile([128, D], f32, tag="o")
                for oc in range(2):
                    ps_o = psO.tile([128, 512], f32, tag="po")
                    for dvb in range(2):
                        nc.tensor.matmul(
                            ps_o[:],
                            ctxT_t[dvb][:, qb * 128:(qb + 1) * 128],
                            wo_t[dvb][:, oc * 512:(oc + 1) * 512],
                            start=(dvb == 0), stop=(dvb == 1))
                    nc.vector.tensor_copy(ob[:, oc * 512:(oc + 1) * 512],
                                          ps_o[:])
                nc.sync.dma_start(outp[qb * 128:(qb + 1) * 128, :], ob[:])

    nc.compile()
    return nc


def kernel(query, key, value, Wq, bq, Wk, bk, Wv, bv, Wo, bo, **trace_kw):
    global _compiled
    if _compiled is None:
        _compiled = _build()
    nc = _compiled

    query = np.asarray(query, np.float32)
    key = np.asarray(key, np.float32)
    value = np.asarray(value, np.float32)
    Wq, Wk, Wv, Wo = (np.asarray(w, np.float32) for w in (Wq, Wk, Wv, Wo))
    bq, bk, bv, bo = (np.asarray(x, np.float32) for x in (bq, bk, bv, bo))

    ones = np.ones((1, S), np.float32)
    xqT_b = [np.ascontiguousarray(query[b].T) for b in range(B)]
    xkT_b = [np.ascontiguousarray(key[b].T) for b in range(B)]
    xvT_b = [np.concatenate([value[b].T, ones], axis=0) for b in range(B)]

    in_maps = []
    for c in range(NCORES):
        b, hg = c // GPB, c % GPB
        sl = slice(hg * DV, (hg + 1) * DV)
        in_maps.append({
            "xqT": xqT_b[b], "xkT": xkT_b[b], "xvT": xvT_b[b],
            "wqT": np.ascontiguousarray(Wq[sl, :].T),
            "wkT": np.ascontiguousarray(Wk[sl, :].T),
            "wvT": np.concatenate(
                [np.ascontiguousarray(Wv[sl, :].T), bv[sl][None, :]], axis=0),
            "woT": np.ascontiguousarray(Wo[:, sl].T),
            "bq": np.ascontiguousarray(bq[sl].reshape(DV, 1)),
            "bk": np.ascontiguousarray(bk[sl].reshape(DV, 1)),
        })

    res = run_bass_kernel_spmd(nc, in_maps, core_ids=list(range(NCORES)),
                               **trace_kw)

    attn = np.empty((B, H, S, S), np.float32)
    out = np.zeros((B, S, D), np.float32)
    for c in range(NCORES):
        b, hg = c // GPB, c % GPB
        attn[b, hg * HPC:(hg + 1) * HPC] = res.results[c]["attn4"]
        out[b] += res.results[c]["outp"]
    out += bo[None, None, :]
    if trace_kw:
        return (out, attn), res
    return out, attn
